# revision 10
# baseline (speedup 1.0000x reference)
"""Fused multi-head attention (2 heads, RoPE-across-heads) on 8 Trainium2 NeuronCores.

Reference computation (per batch b of 4, seq 2048, dim 2048):
    qkv = x @ wqkv; rope mixes the two heads; scores = q'k'^T/32; softmax;
    out = (attn @ v) @ wout + bout

Sharding: core c owns (batch = c//2, seq-half = c%2) -> 1024 query rows.
Each core projects q/k/v for its own 1024 rows, ropes q/k, AllGathers
k'/v within the (2c, 2c+1) pair, runs attention for its rows against the
full 2048-seq k'/v, and applies the output projection for its rows.

v2 pipeline notes (post-trace): the two 4MB pair-AllGathers run at only
~37 GB/s for ~110us each and starve the weight-prefetch DMA stream,
stalling the PE ~86us total.  Fixes: deep weight prefetch rings (wv
fully resident before v-proj, q wst ring shared with k's), AllGathers
split in two 2MB chunks triggered at phase midpoints (k staging layout
permuted so chunks are contiguous), x load split across two DMA
queues, attention k/v tiles double-buffered across heads, wout
prefetched on the idle gpsimd queue.

On-device layouts (partition dim first):
    xT    [dim, rows]      - rhs/stationary for projections
    q'T   [2048, rows]     - head-dim on partitions (chunked [128,16,1024])
    k_in  [2048, rows]     - roped k^T, rows permuted: [h0 d0-511, h1 d0-511,
                             h0 d512-1023, h1 d512-1023] so each half is
                             contiguous for the chunked AllGather
    v     [rows, 2048]     - natural; gathered row-halves into v_g1/v_g2
    P^T   [seq_j, rows]    - exp(scores^T), bf16
    aoT   [2048, rows]     - unnormalized attn-out^T, normalized on write

Softmax skips max-subtraction: scores = q'.k'/32 ~ N(0,1), |scores| < ~8,
so exp is safe in f32 (verified against the reference distribution).
"""

import os
import sys

import numpy as np

if "/opt/trn_rl_repo" not in sys.path:
    sys.path.insert(0, "/opt/trn_rl_repo")

import ml_dtypes

# ---------------------------------------------------------------- constants
B, S, D = 4, 2048, 2048          # batch, seq, model dim
H, HD = 2, 1024                  # heads, head dim
R = 1024                         # query rows per core
N_CORES = 8
SCALE = 1.0 / 32.0               # HD ** -0.5

_NC_CACHE = {}
LAST_RESULT = {}

PAIRS = [[0, 1], [2, 3], [4, 5], [6, 7]]


def _build():
    import concourse.bass as bass
    import concourse.tile as tile
    from concourse import bacc, mybir

    F32 = mybir.dt.float32
    F16 = mybir.dt.float16
    BF = mybir.dt.bfloat16
    Exp = mybir.ActivationFunctionType.Exp

    NRB = R // 512                             # 512-row blocks for q / attention

    nc = bacc.Bacc("TRN2", target_bir_lowering=False, debug=False,
                   num_devices=N_CORES)

    xT = nc.dram_tensor("xT", [D, R], BF, kind="ExternalInput").ap()
    wqkv = nc.dram_tensor("wqkv", [D, 3 * D], BF, kind="ExternalInput").ap()
    wout = nc.dram_tensor("wout", [D, D], BF, kind="ExternalInput").ap()
    cost = nc.dram_tensor("cost", [512, R], F16, kind="ExternalInput").ap()
    sint = nc.dram_tensor("sint", [512, R], F16, kind="ExternalInput").ap()
    bias = nc.dram_tensor("bias", [1, D], F32, kind="ExternalInput").ap()
    out = nc.dram_tensor("out", [R, D], F32, kind="ExternalOutput").ap()

    wq_r = wqkv.rearrange("(c p) m -> p c m", p=128)    # [128, 16, 6144]
    x_r = xT.rearrange("(c p) r -> p c r", p=128)       # [128, 16, R]
    c_r = cost.rearrange("(c p) r -> p c r", p=128)     # [128, 4, R]
    s_r = sint.rearrange("(c p) r -> p c r", p=128)

    def bcast_ap(src_ap, nparts, width):
        return bass.AP(tensor=src_ap.tensor, offset=src_ap.offset,
                       ap=[[0, nparts], [1, width]])

    with tile.TileContext(nc) as tc:
        with (
            tc.tile_pool(name="persist", bufs=1) as persist,
            tc.tile_pool(name="psum", bufs=6, space="PSUM") as psp,
            tc.tile_pool(name="dram", bufs=1, space="DRAM") as dram,
        ):
            # ------------------------------------------- persistent buffers
            qT_sb = persist.tile([128, 16, R], BF, tag="qT")
            bias_sb = persist.tile([128, D], F32, tag="bias")
            ones_sb = persist.tile([128, 1], BF, tag="ones")
            nc.vector.memset(ones_sb, 1.0)

            # DRAM scratch.  k_in rows permuted: row' = half*1024 + head*512
            # + (d % 512) for head-dim d, so halves are contiguous slabs.
            k_in = dram.tile([D, R], BF, tag="k_in")
            v_in = dram.tile([R, D], BF, tag="v_in")
            # gathered halves: [rank0 slab, rank1 slab]
            k_g1 = dram.tile([D, R], BF, tag="k_g1")   # d 0-511 both heads
            k_g2 = dram.tile([D, R], BF, tag="k_g2")   # d 512-1023 both heads
            v_g1 = dram.tile([S // 2, D], BF, tag="v_g1")  # local rows 0-511
            v_g2 = dram.tile([S // 2, D], BF, tag="v_g2")  # local rows 512-1023

            # =================================================== projections
            with (
                tc.tile_pool(name="proj", bufs=1) as proj,
                tc.tile_pool(name="projs", bufs=1) as projs,
            ):
                # first stationary weights for the k projection (scalar q)
                w_first = []
                for c in (0, 8):
                    wt = projs.tile([128, 16, 128], BF, tag="wst", bufs=8)
                    for hh in (0, 8):
                        nc.scalar.dma_start(
                            out=wt[:, hh:hh + 8, :],
                            in_=wq_r[:, hh:hh + 8, D + c * 128:D + (c + 1) * 128])
                    w_first.append(wt)
                # x as four 1MB tiles (dep tracking is tile-granular, so the
                # first matmul chain starts after ~1MB instead of 4MB)
                x_parts = []
                for xp in range(4):
                    xt = proj.tile([128, 4, R], BF, tag="x", bufs=4)
                    nc.sync.dma_start(out=xt, in_=x_r[:, xp * 4:(xp + 1) * 4, :])
                    x_parts.append(xt)

                def x_ap(kc, rs):
                    return x_parts[kc // 4][:, kc % 4, rs]

                # rope tables: chunk 0 first (needed by the first rope); the
                # rest on the gpsimd queue, off the weight stream's path
                cs_sb = proj.tile([128, 4, R], F16, tag="cs")
                ss_sb = proj.tile([128, 4, R], F16, tag="ss")
                nc.scalar.dma_start(out=cs_sb[:, 0, :], in_=c_r[:, 0, :])
                nc.scalar.dma_start(out=ss_sb[:, 0, :], in_=s_r[:, 0, :])
                nc.gpsimd.dma_start(out=cs_sb[:, 1:, :], in_=c_r[:, 1:, :])
                nc.gpsimd.dma_start(out=ss_sb[:, 1:, :], in_=s_r[:, 1:, :])
                # v-projection weight tiles: allocated now, loaded paced into
                # the k-proj weight stream (see on_c_done) so the early HBM
                # window isn't oversubscribed
                wv_tiles = []
                for _vc in range(4):
                    wv_t = projs.tile([128, 16, 512], BF, tag="wv", bufs=4)
                    wv_tiles.append(wv_t)

                def load_wv(vc):
                    wv = wv_tiles[vc]
                    for kc in range(0, 16, 4):
                        nc.scalar.dma_start(
                            out=wv[:, kc:kc + 4, :],
                            in_=wq_r[:, kc:kc + 4,
                                     2 * D + vc * 512:2 * D + (vc + 1) * 512])

                def load_wst(col0, cc0, dma_eng):
                    wt = projs.tile([128, 16, 128], BF, tag="wst", bufs=8)
                    for hh in (0, 8):
                        dma_eng.dma_start(
                            out=wt[:, hh:hh + 8, :],
                            in_=wq_r[:, hh:hh + 8,
                                     col0 + cc0 * 128:col0 + (cc0 + 1) * 128])
                    return wt

                def qk_proj(col0, nrb, emit, dma_eng, preloaded=None,
                            on_c_done=None):
                    """Project+rope cols [col0, col0+2048) of wqkv.

                    emit(c, rb, apA, apB): receive bf16 [128,512] rope outputs
                    for col-chunk c (head0) and c+8 (head1), row block rb."""
                    for c in range(8):
                        if c == 0 and preloaded is not None:
                            w1, w2 = preloaded
                        else:
                            w1 = load_wst(col0, c, dma_eng)
                            w2 = load_wst(col0, c + 8, dma_eng)
                        for rb in range(nrb):
                            rs = slice(rb * 512, (rb + 1) * 512)
                            ps1 = psp.tile([128, 512], F32, tag="mm")
                            ps2 = psp.tile([128, 512], F32, tag="mm")
                            for kc in range(16):
                                nc.tensor.matmul(ps1, w1[:, kc, :], x_ap(kc, rs),
                                                 start=kc == 0, stop=kc == 15)
                            for kc in range(16):
                                nc.tensor.matmul(ps2, w2[:, kc, :], x_ap(kc, rs),
                                                 start=kc == 0, stop=kc == 15)
                            cosv = cs_sb[:, c % 4, rs]
                            sinv = ss_sb[:, c % 4, rs]
                            t1 = projs.tile([128, 512], F32, tag="rt", bufs=4)
                            t2 = projs.tile([128, 512], F32, tag="rt", bufs=4)
                            outA = projs.tile([128, 512], BF, tag="ro", bufs=4)
                            outB = projs.tile([128, 512], BF, tag="ro", bufs=4)
                            nc.vector.tensor_mul(t1, ps1, cosv)
                            nc.vector.tensor_mul(t2, ps2, sinv)
                            nc.vector.tensor_sub(outA, t1, t2)
                            nc.vector.tensor_mul(t1, ps2, cosv)
                            nc.vector.tensor_mul(t2, ps1, sinv)
                            nc.vector.tensor_add(outB, t1, t2)
                            emit(c, rb, outA, outB)
                        if on_c_done is not None:
                            on_c_done(c)

                # ---- k projection + rope -> permuted k_in, chunked AllGather
                def emit_k(c, rb, apA, apB):
                    rs = slice(rb * 512, (rb + 1) * 512)
                    half, cc = c // 4, c % 4
                    base = half * 1024
                    nc.gpsimd.dma_start(
                        out=k_in[base + cc * 128:base + (cc + 1) * 128, rs],
                        in_=apA)
                    nc.gpsimd.dma_start(
                        out=k_in[base + 512 + cc * 128:base + 512 + (cc + 1) * 128, rs],
                        in_=apB)

                def k_ag(c):
                    if c == 3:
                        nc.gpsimd.collective_compute(
                            "AllGather", bass.mybir.AluOpType.bypass,
                            replica_groups=PAIRS,
                            ins=[k_in[0:1024, :].opt()], outs=[k_g1.opt()])
                    elif c == 7:
                        nc.gpsimd.collective_compute(
                            "AllGather", bass.mybir.AluOpType.bypass,
                            replica_groups=PAIRS,
                            ins=[k_in[1024:2048, :].opt()], outs=[k_g2.opt()])
                    if 2 <= c <= 5:
                        load_wv(c - 2)

                qk_proj(D, NRB, emit_k, nc.scalar, preloaded=w_first,
                        on_c_done=k_ag)

                # ---- v projection (natural layout), row-half outer so each
                # half AllGathers while the other half computes
                for half in range(2):
                    for vc in range(4):
                        wv = wv_tiles[vc]
                        for rr in range(half * 4, half * 4 + 4):
                            ps = psp.tile([128, 512], F32, tag="mm")
                            for kc in range(16):
                                nc.tensor.matmul(
                                    ps, x_ap(kc, slice(rr * 128, (rr + 1) * 128)),
                                    wv[:, kc, :], start=kc == 0, stop=kc == 15)
                            vt = projs.tile([128, 512], BF, tag="vo", bufs=4)
                            nc.scalar.copy(vt, ps)
                            nc.scalar.dma_start(
                                out=v_in[rr * 128:(rr + 1) * 128,
                                         vc * 512:(vc + 1) * 512],
                                in_=vt)
                    vg = v_g1 if half == 0 else v_g2
                    nc.gpsimd.collective_compute(
                        "AllGather", bass.mybir.AluOpType.bypass,
                        replica_groups=PAIRS,
                        ins=[v_in[half * 512:(half + 1) * 512, :].opt()],
                        outs=[vg.opt()])

                # ---- q projection + rope -> qT_sb (resident)
                def emit_q(c, rb, apA, apB):
                    rs = slice(rb * 512, (rb + 1) * 512)
                    nc.vector.tensor_copy(qT_sb[:, c, rs], apA)
                    nc.vector.tensor_copy(qT_sb[:, c + 8, rs], apB)

                qk_proj(0, NRB, emit_q, nc.sync)

            # ====================================== attention + output proj
            with tc.tile_pool(name="attn", bufs=1) as attn:
                aoT_sb = attn.tile([128, 16, R], BF, tag="aoT")
                nc.gpsimd.dma_start(out=bias_sb, in_=bcast_ap(bias, 128, D))
                for hi in range(H):
                    # k^T halves: kTa = head dims 0-511, kTb = 512-1023;
                    # ring of 3 so the next head's kTa prefetches early.
                    kTa = attn.tile([128, 4, S], BF, tag="kT", bufs=2)
                    kTb = attn.tile([128, 4, S], BF, tag="kT", bufs=2)
                    for sh in range(2):
                        nc.scalar.dma_start(
                            out=kTa[:, :, sh * R:(sh + 1) * R],
                            in_=k_g1[sh * 1024 + hi * 512:sh * 1024 + (hi + 1) * 512,
                                     :].rearrange("(c p) r -> p c r", p=128))
                        nc.scalar.dma_start(
                            out=kTb[:, :, sh * R:(sh + 1) * R],
                            in_=k_g2[sh * 1024 + hi * 512:sh * 1024 + (hi + 1) * 512,
                                     :].rearrange("(c p) r -> p c r", p=128))
                    # v rows for this head: global key chunks jc 0..15 map to
                    # [v_g1 sh0, v_g2 sh0, v_g1 sh1, v_g2 sh1] quarters.
                    v_sb = attn.tile([128, 16, HD], BF, tag="vh")
                    for quarter in range(4):
                        vg = v_g1 if quarter % 2 == 0 else v_g2
                        sh = quarter // 2
                        nc.scalar.dma_start(
                            out=v_sb[:, quarter * 4:(quarter + 1) * 4, :],
                            in_=vg[sh * 512:(sh + 1) * 512,
                                   hi * HD:(hi + 1) * HD].rearrange(
                                "(c p) m -> p c m", p=128))
                    for rb in range(NRB):
                        rs = slice(rb * 512, (rb + 1) * 512)
                        PT = attn.tile([128, 16, 512], BF, tag="PT", bufs=1)
                        for jc in range(16):
                            ps = psp.tile([128, 512], F32, tag="mm")
                            for dc in range(8):
                                kt = kTa if dc < 4 else kTb
                                nc.tensor.matmul(
                                    ps, kt[:, dc % 4, jc * 128:(jc + 1) * 128],
                                    qT_sb[:, hi * 8 + dc, rs],
                                    start=dc == 0, stop=dc == 7)
                            nc.scalar.activation(PT[:, jc, :], ps, Exp, scale=SCALE)
                        # row sums via ones-matmul, then reciprocal broadcast
                        sps = psp.tile([1, 512], F32, tag="sum", bufs=2)
                        for jc in range(16):
                            nc.tensor.matmul(sps, ones_sb, PT[:, jc, :],
                                             start=jc == 0, stop=jc == 15)
                        rec = attn.tile([1, 512], F32, tag="rec", bufs=2)
                        nc.vector.reciprocal(rec, sps)
                        rec_d = dram.tile([1, 512], F32, tag="rec_d", bufs=2)
                        nc.sync.dma_start(out=rec_d, in_=rec)
                        rec_b = attn.tile([128, 512], F32, tag="rec_b", bufs=2)
                        nc.sync.dma_start(out=rec_b, in_=bcast_ap(rec_d, 128, 512))
                        for m in range(8):
                            pa = psp.tile([128, 512], F32, tag="mm")
                            for jc in range(16):
                                nc.tensor.matmul(
                                    pa, v_sb[:, jc, m * 128:(m + 1) * 128],
                                    PT[:, jc, :], start=jc == 0, stop=jc == 15)
                            nc.vector.tensor_mul(aoT_sb[:, hi * 8 + m, rs], pa, rec_b)

                # ---------------------------------------- output projection
                wout_r = wout.rearrange("(c p) m -> p c m", p=128)
                for cc in range(4):
                    wo = attn.tile([128, 16, 512], BF, tag="wo", bufs=2)
                    for dc in range(0, 16, 2):
                        nc.gpsimd.dma_start(
                            out=wo[:, dc:dc + 2, :],
                            in_=wout_r[:, dc:dc + 2, cc * 512:(cc + 1) * 512])
                    for rr in range(R // 128):
                        r0 = rr * 128
                        ps = psp.tile([128, 512], F32, tag="mm")
                        for dc in range(16):
                            nc.tensor.matmul(ps, aoT_sb[:, dc, r0:r0 + 128],
                                             wo[:, dc, :],
                                             start=dc == 0, stop=dc == 15)
                        ot = attn.tile([128, 512], F32, tag="ot", bufs=4)
                        nc.vector.tensor_add(ot, ps, bias_sb[:, cc * 512:(cc + 1) * 512])
                        nc.gpsimd.dma_start(
                            out=out[r0:r0 + 128, cc * 512:(cc + 1) * 512], in_=ot)

    nc.compile()
    return nc


def _get_nc():
    if "v2" not in _NC_CACHE:
        _NC_CACHE["v2"] = _build()
    return _NC_CACHE["v2"]


def _rope_tables():
    inv_freq = 1.0 / (10000.0 ** (np.arange(0, HD, 2, dtype=np.float32) / HD))
    t = np.arange(S, dtype=np.float32)
    freqs = t[:, None] * inv_freq[None, :]          # (S, 512)
    return np.cos(freqs).astype(np.float32), np.sin(freqs).astype(np.float32)


def kernel(x, wqkv, wout, bout):
    from concourse.bass_utils import run_bass_kernel_spmd

    bf16 = ml_dtypes.bfloat16
    x = np.asarray(x, dtype=np.float32)
    wqkv_b = np.ascontiguousarray(np.asarray(wqkv, dtype=np.float32)).astype(bf16)
    wout_b = np.ascontiguousarray(np.asarray(wout, dtype=np.float32)).astype(bf16)
    bout_f = np.asarray(bout, dtype=np.float32).reshape(1, D)
    cos_h, sin_h = _rope_tables()                   # (S, 512) f32
    cosT = np.ascontiguousarray(cos_h.T)            # (512, S)
    sinT = np.ascontiguousarray(sin_h.T)

    nc = _get_nc()

    in_maps = []
    for c in range(N_CORES):
        bi, half = c // 2, c % 2
        rows = slice(half * R, (half + 1) * R)
        xT_own = np.ascontiguousarray(x[bi, rows, :].T).astype(bf16)
        in_maps.append({
            "wqkv": wqkv_b,
            "wout": wout_b,
            "bias": bout_f,
            "xT": xT_own,
            "cost": np.ascontiguousarray(cosT[:, rows]).astype(np.float16),
            "sint": np.ascontiguousarray(sinT[:, rows]).astype(np.float16),
        })

    trace = os.environ.get("KERNEL_TRACE", "0") == "1"
    res = run_bass_kernel_spmd(nc, in_maps, list(range(N_CORES)), trace=trace)
    if trace:
        LAST_RESULT["exec_time_ns"] = res.exec_time_ns
        LAST_RESULT["mean_exec_time_ns"] = res.mean_exec_time_ns

    out_full = np.empty((B, S, D), np.float32)
    for c in range(N_CORES):
        bi, half = c // 2, c % 2
        out_full[bi, half * R:(half + 1) * R, :] = res.results[c]["out"]
    return out_full
